# revision 1
# baseline (speedup 1.0000x reference)
"""Trainium2 Bass kernel: 4096x4096 fp32 image, 9x9 valid cross-correlation + bias.

Strategy
--------
Row-shard the image across 8 NeuronCores (spatial data parallel, 8-row halo
given to each core from the host; kernel/bias replicated). Per core the conv
is computed on the tensor engine as banded matmuls:

  out[m, n] = sum_dj sum_k B_dj[k, m] * X[r0+k, c0+dj+n]

with B_dj[k, m] = kern[k-m, dj] for 0 <= k-m < 9 (else 0), a 128x120 banded
"Toeplitz" stationary operand built on the host. One PSUM accumulation group
of 9 matmuls (one per kernel column dj) covers all 81 taps for a
[120 out-rows x 512 out-cols] tile. rhs slices are plain offset views of a
single SBUF tile of X in natural layout (partition = image row).

float32r operands give 1 cycle/row on the PE (4x over fp32) at ~1e-4
scale-relative accuracy; operands are rounded to f32r by DVE copies as the
BIR verifier requires.
"""

import numpy as np

H, W = 4096, 4096
KH, KW = 9, 9
NCORES = 8
OH, OW = H - KH + 1, W - KW + 1  # 4088, 4088
RPC = OH // NCORES  # 511 output rows per core
IN_ROWS = RPC + KH - 1  # 519 input rows per core (8-row halo)
MB = 120  # output rows per full row-block (128 input rows - 8)
NB = 512  # output cols per tile (one PSUM bank of fp32)

# (input row offset, input rows, output rows) per row block: 4x120 + 31 = 511
ROW_BLOCKS = [(0, 128, 120), (120, 128, 120), (240, 128, 120), (360, 128, 120),
              (480, 39, 31)]
# (col offset, output cols) per column tile: 7x512 + 504 = 4088
COL_TILES = [(512 * ct, 512 if ct < 7 else OW - 512 * 7) for ct in range(8)]


def _build_nc(repeat=1):
    import concourse.bacc as bacc
    import concourse.mybir as mybir
    import concourse.tile as tile

    F32 = mybir.dt.float32
    F32R = mybir.dt.float32r

    nc = bacc.Bacc("TRN2", target_bir_lowering=False, debug=False)
    Xs = nc.dram_tensor("Xs", [IN_ROWS, W], F32, kind="ExternalInput")
    Bm = nc.dram_tensor("Bm", [128, KW * MB], F32, kind="ExternalInput")
    Bc = nc.dram_tensor("Bc", [128, 1], F32, kind="ExternalInput")
    O = nc.dram_tensor("O", [RPC, OW], F32, kind="ExternalOutput")

    with tile.TileContext(nc) as tc:
        with (
            tc.tile_pool(name="const", bufs=1) as cpool,
            tc.tile_pool(name="xp", bufs=3) as xp,
            tc.tile_pool(name="xrp", bufs=3) as xrp,
            tc.tile_pool(name="op", bufs=3) as op,
            tc.tile_pool(name="pp", bufs=4, space="PSUM") as pp,
        ):
            b_f32 = cpool.tile([128, KW * MB], F32)
            nc.sync.dma_start(b_f32[:], Bm[:])
            b_r = cpool.tile([128, KW * MB], F32R)
            nc.vector.tensor_copy(b_r[:], b_f32[:])
            bias_sb = cpool.tile([128, 1], F32)
            nc.sync.dma_start(bias_sb[:], Bc[:])

            for _ in range(repeat):
                for r0, kb, mb in ROW_BLOCKS:
                    for c0, nb in COL_TILES:
                        xt = xp.tile([128, NB + 8], F32, tag="x")
                        nc.sync.dma_start(
                            xt[:kb, : nb + 8], Xs[r0 : r0 + kb, c0 : c0 + nb + 8]
                        )
                        xr = xrp.tile([128, NB + 8], F32R, tag="xr")
                        nc.vector.tensor_copy(xr[:kb, : nb + 8], xt[:kb, : nb + 8])
                        ps = pp.tile([128, NB], F32, tag="ps")
                        for dj in range(KW):
                            nc.tensor.matmul(
                                ps[:mb, :nb],
                                b_r[:kb, dj * MB : dj * MB + mb],
                                xr[:kb, dj : dj + nb],
                                start=(dj == 0),
                                stop=(dj == KW - 1),
                            )
                        ot = op.tile([128, NB], F32, tag="o")
                        nc.vector.tensor_scalar_add(
                            ot[:mb, :nb], ps[:mb, :nb], bias_sb[:mb, 0:1]
                        )
                        nc.sync.dma_start(O[r0 : r0 + mb, c0 : c0 + nb], ot[:mb, :nb])

    nc.compile()
    return nc


def _host_inputs(X, kern, bias):
    """Build per-core input maps: sharded X with halo + replicated band/bias."""
    X = np.ascontiguousarray(np.asarray(X, dtype=np.float32))
    kern = np.asarray(kern, dtype=np.float32)
    bias = np.asarray(bias, dtype=np.float32)

    Bm = np.zeros((128, KW * MB), np.float32)
    m = np.arange(MB)
    for dj in range(KW):
        for d in range(KH):
            Bm[m + d, dj * MB + m] = kern[d, dj]
    Bc = np.full((128, 1), bias[0], np.float32)

    return [
        {"Xs": X[RPC * c : RPC * c + IN_ROWS, :], "Bm": Bm, "Bc": Bc}
        for c in range(NCORES)
    ]


_NC_CACHE = {}


def _get_nc(repeat=1):
    if repeat not in _NC_CACHE:
        _NC_CACHE[repeat] = _build_nc(repeat)
    return _NC_CACHE[repeat]


def kernel(X, kernel, bias):
    from concourse.bass_utils import run_bass_kernel_spmd

    nc = _get_nc()
    in_maps = _host_inputs(X, kernel, bias)
    res = run_bass_kernel_spmd(nc, in_maps, core_ids=list(range(NCORES)))
    out = np.empty((OH, OW), np.float32)
    for c in range(NCORES):
        out[RPC * c : RPC * (c + 1), :] = res.results[c]["O"]
    return out
